# revision 3
# baseline (speedup 1.0000x reference)
"""Fused int8 dequant -> causal mask -> softmax -> int8 requant on 8 TRN2 cores.

Problem: x_q [B=4, H=16, S=1024, S] int8, per-(head,row) scales sx/so [H*S] f32.
  out = int8(clip(round(softmax(causal_mask(x_q * sx)) / so), -128, 127))

Sharding: 2 heads per core (data parallel over 64 independent (b, h) planes).
Rows live on partitions; softmax runs along the free dim. For each (h, t)
row-tile of 128 rows only cols [0, W=(t+1)*128) are moved (causal packing).
Host premasks x (strict upper triangle zeroed) so masked lanes contribute
exp(0)=1 to row sums, corrected by the compile-time constant (127 - p).

v2 engine assignment (vs v1's ACT-heavy accum design):
  - ACT: one merged Exp per (h,t) over [128, B*W] for most tiles (no accum,
    no per-b split) -> ~37us busy. Only ACT_SUM_TILES largest tiles keep the
    per-b exp+accum_out split (sums ride along for +3 instr inits + 4
    readouts).
  - DVE sums for remaining tiles: 2x fp16 fold-in-half twice (all b in one
    strided TT), then one 3D tensor_reduce [P,(B,W/4)] -> [P,B]. 5/8 the
    1x cost.
  - requant: per-b tensor_scalar fp16 -> int16 ((et*r) min 127), which runs
    in the DVE 2x/4x modes (int8 output would force 1x); then ONE merged
    scalar_tensor_tensor packs int16 pairs (hi*256 + lo) across all b.
    Output words hold bytes (y[j], y[W/2+j]) -- the host de-interleaves.
  - GPSIMD: diag-block causal masks (TT with stride-0 tri broadcast) +
    the (sums-corr)*so smalls; DVE does reciprocal_approx_fast.
  - y = round(p/so) >= 0 always (softmax >= 0), so min 127 suffices for the
    clip and the int16 word value is < 32640: no saturation/sign issues.
"""

import contextlib
import ctypes
import os
import sys
import types
from contextlib import ExitStack

import numpy as np

import concourse.bacc as bacc
import concourse.bass as bass
import concourse.tile as tile
from concourse import mybir
from concourse.bass_utils import run_bass_kernel_spmd

B, H, S = 4, 16, 1024
NCORES = 8
HPC = H // NCORES  # heads per core
P = 128
NT = S // P  # row tiles per plane
AF = mybir.ActivationFunctionType
ALU = mybir.AluOpType

# tiles t >= ACT_SUM_T keep per-b exp+accum on ACT; smaller tiles sum on DVE
ACT_SUM_T = 6
# tiles with W >= FOLD_MIN_W use the double-fold before the reduce
FOLD_MIN_W = 256

# packed block offsets: block (h, t) holds [P, B*W] int8, W = (t+1)*P
_BLK = [[None] * NT for _ in range(HPC)]
_off = 0
for _h in range(HPC):
    for _t in range(NT):
        _W = (_t + 1) * P
        _BLK[_h][_t] = (_off, _W)
        _off += P * B * _W
TOTAL = _off  # per-core packed bytes (4718592)

_AXON_SO = "/opt/axon/libaxon_pjrt.so"


def _ensure_ntff_hook():
    """This image's antenv lacks axon_hooks; provide it so trace=True works."""
    if "antenv.axon_hooks" in sys.modules:
        return
    import antenv

    mod = types.ModuleType("antenv.axon_hooks")
    state = {"hook": None}
    mod.set_axon_ntff_profile_hook = lambda h: state.__setitem__("hook", h)
    mod.get_axon_ntff_profile_hook = lambda: state["hook"]
    sys.modules["antenv.axon_hooks"] = mod
    antenv.axon_hooks = mod

    if not os.path.exists(_AXON_SO):
        return
    lib = ctypes.CDLL(_AXON_SO)
    if not hasattr(lib, "axon_start_nrt_profile"):
        return
    lib.axon_start_nrt_profile.argtypes = [ctypes.POINTER(ctypes.c_int64), ctypes.c_size_t]
    lib.axon_start_nrt_profile.restype = ctypes.c_int64
    lib.axon_stop_nrt_profile.argtypes = [ctypes.c_char_p]
    lib.axon_stop_nrt_profile.restype = ctypes.c_int64

    @contextlib.contextmanager
    def _hook(output_dir, device_ids):
        import jax

        jax.devices()
        if device_ids:
            ids = (ctypes.c_int64 * len(device_ids))(*device_ids)
            rc = lib.axon_start_nrt_profile(ids, len(device_ids))
        else:
            rc = lib.axon_start_nrt_profile(None, 0)
        if rc != 0:
            raise RuntimeError(f"axon_start_nrt_profile rc={rc}")
        try:
            yield
        finally:
            n = lib.axon_stop_nrt_profile(str(output_dir).encode())
            print(f"profile: {n} file(s) written to {output_dir}", file=sys.stderr)

    mod.set_axon_ntff_profile_hook(_hook)


_cached_nc = None


def _ap3(t, off, mid_stride, mid_n, inner_n):
    return bass.AP(tensor=t.tensor, offset=t.offset + off,
                   ap=[t.ap[0], [mid_stride, mid_n], [1, inner_n]])


def _build_bass(compile=True):
    nc = bacc.Bacc("TRN2", target_bir_lowering=False, debug=False,
                   num_devices=NCORES)
    x = nc.declare_dram_parameter("x", [TOTAL], mybir.dt.int8, isOutput=False)
    sx = nc.declare_dram_parameter("sx", [P, HPC * NT], mybir.dt.float32, isOutput=False)
    so = nc.declare_dram_parameter("so", [P, HPC * NT], mybir.dt.float32, isOutput=False)
    corr = nc.declare_dram_parameter("corr", [P, 1], mybir.dt.float32, isOutput=False)
    tri = nc.declare_dram_parameter("tri", [P, P], mybir.dt.float16, isOutput=False)
    y = nc.declare_dram_parameter("y", [TOTAL // 2], mybir.dt.int16, isOutput=True)

    with ExitStack() as ctx:
        tc = ctx.enter_context(tile.TileContext(nc))
        singles = ctx.enter_context(tc.tile_pool(name="singles", bufs=1))
        xpool = ctx.enter_context(tc.tile_pool(name="xp", bufs=5))
        epool = ctx.enter_context(tc.tile_pool(name="ep", bufs=3))
        qpool = ctx.enter_context(tc.tile_pool(name="qp", bufs=2))
        ypool = ctx.enter_context(tc.tile_pool(name="yp", bufs=5))
        fpool = ctx.enter_context(tc.tile_pool(name="fp", bufs=2))
        smalls = ctx.enter_context(tc.tile_pool(name="sm", bufs=8))

        sxt = singles.tile([P, HPC * NT], mybir.dt.float32)
        nc.sync.dma_start(sxt[:], sx[:])
        sot = singles.tile([P, HPC * NT], mybir.dt.float32)
        nc.sync.dma_start(sot[:], so[:])
        corrt = singles.tile([P, 1], mybir.dt.float32)
        nc.sync.dma_start(corrt[:], corr[:])
        trit = singles.tile([P, P], mybir.dt.float16)
        nc.sync.dma_start(trit[:], tri[:])

        for h in range(HPC):
            for t in range(NT):
                off, W = _BLK[h][t]
                col = h * NT + t

                xt = xpool.tile([P, B * W], mybir.dt.int8, tag="xt")
                nc.sync.dma_start(
                    xt[:], x[off:off + P * B * W].rearrange("(p n) -> p n", p=P))

                et = epool.tile([P, B * W], mybir.dt.float16, tag="et")
                sums = smalls.tile([P, B], mybir.dt.float32, tag="sums")
                if t >= ACT_SUM_T:
                    # per-b exp with row sums from the ACT accumulator
                    for b in range(B):
                        nc.scalar.activation(et[:, b * W:(b + 1) * W],
                                             xt[:, b * W:(b + 1) * W],
                                             AF.Exp, bias=0.0,
                                             scale=sxt[:, col:col + 1],
                                             accum_out=sums[:, b:b + 1])
                else:
                    # merged exp; sums via fp16 folds + one 3D reduce on DVE
                    nc.scalar.activation(et[:], xt[:], AF.Exp, bias=0.0,
                                         scale=sxt[:, col:col + 1])
                    if W >= FOLD_MIN_W:
                        h1 = W // 2
                        h2 = W // 4
                        f1 = fpool.tile([P, B * (P * NT // 2)], mybir.dt.float16,
                                        tag="f1")
                        nc.vector.tensor_tensor(
                            _ap3(f1, 0, h1, B, h1),
                            _ap3(et, 0, W, B, h1),
                            _ap3(et, h1, W, B, h1), ALU.add)
                        f2 = fpool.tile([P, B * (P * NT // 4)], mybir.dt.float16,
                                        tag="f2")
                        nc.vector.tensor_tensor(
                            _ap3(f2, 0, h2, B, h2),
                            _ap3(f1, 0, h1, B, h2),
                            _ap3(f1, h2, h1, B, h2), ALU.add)
                        nc.vector.tensor_reduce(
                            sums[:], _ap3(f2, 0, h2, B, h2),
                            mybir.AxisListType.X, ALU.add)
                    else:
                        nc.vector.tensor_reduce(
                            sums[:], _ap3(et, 0, W, B, W),
                            mybir.AxisListType.X, ALU.add)

                # zero the masked (strict upper) part of the diagonal block
                # (GPSIMD; one strided TT, tri broadcast via stride-0 dim)
                dz = t * P
                diag = _ap3(et, dz, W, B, P)
                trib = bass.AP(tensor=trit.tensor, offset=trit.offset,
                               ap=[trit.ap[0], [0, B], [1, P]])
                nc.gpsimd.tensor_tensor(diag, diag, trib, ALU.mult)

                # r = 1/((sums - corr) * so): one STT + fast recip on DVE
                rt = smalls.tile([P, B], mybir.dt.float32, tag="rt")
                sob = bass.AP(tensor=sot.tensor, offset=sot.offset + col,
                              ap=[sot.ap[0], [0, B]])
                nc.vector.scalar_tensor_tensor(rt[:], sums[:], corrt[:], sob,
                                               ALU.subtract, ALU.mult)
                nc.vector.reciprocal_approx_fast(rt[:], rt[:])

                # requant: per-b (et*r) min 127 -> int16 (2x/4x), then one
                # merged pack (hi*256 + lo) -> int16 words of two int8 lanes
                q16 = qpool.tile([P, B * W], mybir.dt.int16, tag="q16")
                for b in range(B):
                    nc.vector.tensor_scalar(q16[:, b * W:(b + 1) * W],
                                            et[:, b * W:(b + 1) * W],
                                            rt[:, b:b + 1], 127.0,
                                            ALU.mult, ALU.min)
                hw = W // 2
                yt = ypool.tile([P, B * hw], mybir.dt.int16, tag="yt")
                nc.vector.scalar_tensor_tensor(
                    _ap3(yt, 0, hw, B, hw),
                    _ap3(q16, hw, W, B, hw), 256.0,
                    _ap3(q16, 0, W, B, hw),
                    ALU.mult, ALU.add)

                nc.sync.dma_start(
                    y[off // 2:off // 2 + P * B * hw].rearrange(
                        "(p n) -> p n", p=P), yt[:])
    if compile:
        nc.compile()
    return nc


_tril_mask = None


def _host_prep(x_q, scale_x, scale_out):
    global _tril_mask
    x_q = np.asarray(x_q)
    assert x_q.dtype == np.int8, x_q.dtype
    scale_x = np.asarray(scale_x, dtype=np.float32).reshape(H, S)
    scale_out = np.asarray(scale_out, dtype=np.float32).reshape(H, S)

    if _tril_mask is None:
        _tril_mask = np.tril(np.ones((S, S), dtype=np.int8))
    x_pm = x_q * _tril_mask  # zero the strict upper triangle

    # [P, H, NT]: sxr[p, h, t] = scale_x[h, t*128 + p]
    sxr = scale_x.reshape(H, NT, P).transpose(2, 0, 1)
    sor = scale_out.reshape(H, NT, P).transpose(2, 0, 1)

    corr = (127 - np.arange(P)).astype(np.float32).reshape(P, 1)
    tri = np.tril(np.ones((P, P), dtype=np.float16))

    in_maps = []
    for c in range(NCORES):
        xc = np.empty(TOTAL, np.int8)
        for h in range(HPC):
            hg = c * HPC + h
            for t in range(NT):
                off, W = _BLK[h][t]
                # [B, P, W] -> [P, B, W] flattened
                blk = x_pm[:, hg, t * P:(t + 1) * P, 0:W].transpose(1, 0, 2)
                xc[off:off + P * B * W] = blk.reshape(-1)
        hs = slice(c * HPC, (c + 1) * HPC)
        sxc = np.ascontiguousarray(sxr[:, hs].reshape(P, HPC * NT))
        soc = np.ascontiguousarray(sor[:, hs].reshape(P, HPC * NT))
        in_maps.append({"x": xc, "sx": sxc, "so": soc, "corr": corr, "tri": tri})
    return in_maps


def _host_unpack(results):
    out = np.zeros((B, H, S, S), np.int8)
    for c in range(NCORES):
        yc = np.asarray(results[c]["y"]).view(np.int8)  # [TOTAL] bytes
        for h in range(HPC):
            hg = c * HPC + h
            for t in range(NT):
                off, W = _BLK[h][t]
                hw = W // 2
                # words [P, B, hw] with bytes (lo=y[j], hi=y[hw+j])
                blk = yc[off:off + P * B * W].reshape(P, B, hw, 2)
                dst = out[:, hg, t * P:(t + 1) * P, 0:W]
                dst[:, :, 0:hw] = blk[:, :, :, 0].transpose(1, 0, 2)
                dst[:, :, hw:W] = blk[:, :, :, 1].transpose(1, 0, 2)
    return out


def run(x_q, scale_x, scale_out, trace=False):
    global _cached_nc
    if trace:
        _ensure_ntff_hook()
    if _cached_nc is None:
        _cached_nc = _build_bass()
    in_maps = _host_prep(x_q, scale_x, scale_out)
    res = run_bass_kernel_spmd(_cached_nc, in_maps, core_ids=list(range(NCORES)),
                               trace=trace)
    return _host_unpack(res.results), res


def kernel(x_q, scale_x, scale_out):
    out, _ = run(x_q, scale_x, scale_out,
                 trace=bool(int(os.environ.get("KERNEL_TRACE", "0"))))
    return out


# revision 8
# speedup vs baseline: 1.0733x; 1.0733x over previous
"""Fused int8 dequant -> causal mask -> softmax -> int8 requant on 8 TRN2 cores.

Problem: x_q [B=4, H=16, S=1024, S] int8, per-(head,row) scales sx/so [H*S] f32.
  out = int8(clip(round(softmax(causal_mask(x_q * sx)) / so), -128, 127))

Sharding: 2 heads per core (data parallel over 64 independent (b, h) planes).
Rows live on partitions; softmax runs along the free dim. For each (h, t)
row-tile of 128 rows only cols [0, W=(t+1)*128) are moved (causal packing).
Host premasks x (strict upper triangle zeroed) so masked lanes contribute
exp(0)=1 to row sums, corrected by the compile-time constant (127 - p).

v2 engine assignment (vs v1's ACT-heavy accum design):
  - ACT: one merged Exp per (h,t) over [128, B*W] for most tiles (no accum,
    no per-b split) -> ~37us busy. Only ACT_SUM_TILES largest tiles keep the
    per-b exp+accum_out split (sums ride along for +3 instr inits + 4
    readouts).
  - DVE sums for remaining tiles: 2x fp16 fold-in-half twice (all b in one
    strided TT), then one 3D tensor_reduce [P,(B,W/4)] -> [P,B]. 5/8 the
    1x cost.
  - requant: per-b tensor_scalar fp16 -> int16 ((et*r) min 127), which runs
    in the DVE 2x/4x modes (int8 output would force 1x); then ONE merged
    scalar_tensor_tensor packs int16 pairs (hi*256 + lo) across all b.
    Output words hold bytes (y[j], y[W/2+j]) -- the host de-interleaves.
  - GPSIMD: diag-block causal masks (TT with stride-0 tri broadcast) +
    the (sums-corr)*so smalls; DVE does reciprocal_approx_fast.
  - y = round(p/so) >= 0 always (softmax >= 0), so min 127 suffices for the
    clip and the int16 word value is < 32640: no saturation/sign issues.
"""

import contextlib
import ctypes
import os
import sys
import types
from contextlib import ExitStack

import numpy as np

import concourse.bacc as bacc
import concourse.bass as bass
import concourse.tile as tile
from concourse import mybir
from concourse.bass_utils import run_bass_kernel_spmd

B, H, S = 4, 16, 1024
NCORES = 8
HPC = H // NCORES  # heads per core
P = 128
NT = S // P  # row tiles per plane
AF = mybir.ActivationFunctionType
ALU = mybir.AluOpType

# tiles t >= ACT_SUM_T keep per-b exp+accum on ACT; smaller tiles sum on DVE
ACT_SUM_T = 4
# tiles with W >= FOLD_MIN_W use the double-fold before the reduce
FOLD_MIN_W = 256
# DVE-sum tiles with W >= GPS_FOLD_W run fold1 on GPSIMD instead of DVE
GPS_FOLD_W = 384

# packed block offsets: block (h, t) holds [P, B*W] int8, W = (t+1)*P
_BLK = [[None] * NT for _ in range(HPC)]
_off = 0
for _h in range(HPC):
    for _t in range(NT):
        _W = (_t + 1) * P
        _BLK[_h][_t] = (_off, _W)
        _off += P * B * _W
TOTAL = _off  # per-core packed bytes (4718592)

_AXON_SO = "/opt/axon/libaxon_pjrt.so"


def _ensure_ntff_hook():
    """This image's antenv lacks axon_hooks; provide it so trace=True works."""
    if "antenv.axon_hooks" in sys.modules:
        return
    import antenv

    mod = types.ModuleType("antenv.axon_hooks")
    state = {"hook": None}
    mod.set_axon_ntff_profile_hook = lambda h: state.__setitem__("hook", h)
    mod.get_axon_ntff_profile_hook = lambda: state["hook"]
    sys.modules["antenv.axon_hooks"] = mod
    antenv.axon_hooks = mod

    if not os.path.exists(_AXON_SO):
        return
    lib = ctypes.CDLL(_AXON_SO)
    if not hasattr(lib, "axon_start_nrt_profile"):
        return
    lib.axon_start_nrt_profile.argtypes = [ctypes.POINTER(ctypes.c_int64), ctypes.c_size_t]
    lib.axon_start_nrt_profile.restype = ctypes.c_int64
    lib.axon_stop_nrt_profile.argtypes = [ctypes.c_char_p]
    lib.axon_stop_nrt_profile.restype = ctypes.c_int64

    @contextlib.contextmanager
    def _hook(output_dir, device_ids):
        import jax

        jax.devices()
        if device_ids:
            ids = (ctypes.c_int64 * len(device_ids))(*device_ids)
            rc = lib.axon_start_nrt_profile(ids, len(device_ids))
        else:
            rc = lib.axon_start_nrt_profile(None, 0)
        if rc != 0:
            raise RuntimeError(f"axon_start_nrt_profile rc={rc}")
        try:
            yield
        finally:
            n = lib.axon_stop_nrt_profile(str(output_dir).encode())
            print(f"profile: {n} file(s) written to {output_dir}", file=sys.stderr)

    mod.set_axon_ntff_profile_hook(_hook)


_cached_nc = None


def _ap3(t, off, mid_stride, mid_n, inner_n):
    return bass.AP(tensor=t.tensor, offset=t.offset + off,
                   ap=[t.ap[0], [mid_stride, mid_n], [1, inner_n]])


def _stt_shift_or(nc, out32, hi32, lo32):
    """out32 = (hi32 << 8) | lo32 -- bitvec STT needs an int32 immediate,
    which the bass wrapper can't express (it lowers floats as fp32)."""
    eng = nc.vector
    return eng.add_instruction(
        mybir.InstTensorScalarPtr(
            name=nc.get_next_instruction_name(),
            is_scalar_tensor_tensor=True,
            op0=mybir.AluOpType.logical_shift_left,
            op1=mybir.AluOpType.bitwise_or,
            ins=[eng.lower_ap(hi32),
                 mybir.ImmediateValue(dtype=mybir.dt.int32, value=8),
                 eng.lower_ap(lo32)],
            outs=[eng.lower_ap(out32)],
        ))


def _build_bass(compile=True):
    nc = bacc.Bacc("TRN2", target_bir_lowering=False, debug=False,
                   num_devices=NCORES)
    x = nc.declare_dram_parameter("x", [TOTAL], mybir.dt.int8, isOutput=False)
    sx = nc.declare_dram_parameter("sx", [P, HPC * NT], mybir.dt.float32, isOutput=False)
    so = nc.declare_dram_parameter("so", [P, HPC * NT], mybir.dt.float32, isOutput=False)
    corr = nc.declare_dram_parameter("corr", [P, 1], mybir.dt.float32, isOutput=False)
    tri = nc.declare_dram_parameter("tri", [P, P], mybir.dt.float16, isOutput=False)
    y = nc.declare_dram_parameter("y", [TOTAL // 2], mybir.dt.int16, isOutput=True)

    with ExitStack() as ctx:
        tc = ctx.enter_context(tile.TileContext(nc))
        singles = ctx.enter_context(tc.tile_pool(name="singles", bufs=1))
        xpool = ctx.enter_context(tc.tile_pool(name="xp", bufs=5))
        epool = ctx.enter_context(tc.tile_pool(name="ep", bufs=3))
        qpool = ctx.enter_context(tc.tile_pool(name="qp", bufs=2))
        ypool = ctx.enter_context(tc.tile_pool(name="yp", bufs=5))
        fpool = ctx.enter_context(tc.tile_pool(name="fp", bufs=2))
        smalls = ctx.enter_context(tc.tile_pool(name="sm", bufs=8))

        sxt = singles.tile([P, HPC * NT], mybir.dt.float32)
        nc.sync.dma_start(sxt[:], sx[:])
        sot = singles.tile([P, HPC * NT], mybir.dt.float32)
        nc.sync.dma_start(sot[:], so[:])
        corrt = singles.tile([P, 1], mybir.dt.float32)
        nc.sync.dma_start(corrt[:], corr[:])
        trit = singles.tile([P, P], mybir.dt.float16)
        nc.sync.dma_start(trit[:], tri[:])

        for h in range(HPC):
            for t in range(NT):
                off, W = _BLK[h][t]
                col = h * NT + t

                xt = xpool.tile([P, B * W], mybir.dt.int8, tag="xt")
                nc.sync.dma_start(
                    xt[:], x[off:off + P * B * W].rearrange("(p n) -> p n", p=P))

                et = epool.tile([P, B * W], mybir.dt.float16, tag="et")
                sums = smalls.tile([P, B], mybir.dt.float32, tag="sums")
                if t >= ACT_SUM_T:
                    # per-b exp with row sums from the ACT accumulator
                    for b in range(B):
                        nc.scalar.activation(et[:, b * W:(b + 1) * W],
                                             xt[:, b * W:(b + 1) * W],
                                             AF.Exp, bias=0.0,
                                             scale=sxt[:, col:col + 1],
                                             accum_out=sums[:, b:b + 1])
                else:
                    # merged exp; sums via fp16 folds + one 3D reduce on DVE
                    nc.scalar.activation(et[:], xt[:], AF.Exp, bias=0.0,
                                         scale=sxt[:, col:col + 1])
                    if W >= FOLD_MIN_W:
                        h1 = W // 2
                        h2 = W // 4
                        f1 = fpool.tile([P, B * (P * NT // 2)], mybir.dt.float16,
                                        tag="f1")
                        feng = nc.gpsimd if W >= GPS_FOLD_W else nc.vector
                        feng.tensor_tensor(
                            _ap3(f1, 0, h1, B, h1),
                            _ap3(et, 0, W, B, h1),
                            _ap3(et, h1, W, B, h1), ALU.add)
                        f2 = fpool.tile([P, B * (P * NT // 4)], mybir.dt.float16,
                                        tag="f2")
                        nc.vector.tensor_tensor(
                            _ap3(f2, 0, h2, B, h2),
                            _ap3(f1, 0, h1, B, h2),
                            _ap3(f1, h2, h1, B, h2), ALU.add)
                        nc.vector.tensor_reduce(
                            sums[:], _ap3(f2, 0, h2, B, h2),
                            mybir.AxisListType.X, ALU.add)
                    else:
                        nc.vector.tensor_reduce(
                            sums[:], _ap3(et, 0, W, B, W),
                            mybir.AxisListType.X, ALU.add)

                # zero the masked (strict upper) part of the diagonal block
                # (GPSIMD; one strided TT, tri broadcast via stride-0 dim)
                dz = t * P
                diag = _ap3(et, dz, W, B, P)
                trib = bass.AP(tensor=trit.tensor, offset=trit.offset,
                               ap=[trit.ap[0], [0, B], [1, P]])
                nc.gpsimd.tensor_tensor(diag, diag, trib, ALU.mult)

                # r = 1/((sums - corr) * so): one STT + fast recip on DVE
                rt = smalls.tile([P, B], mybir.dt.float32, tag="rt")
                sob = bass.AP(tensor=sot.tensor, offset=sot.offset + col,
                              ap=[sot.ap[0], [0, B]])
                nc.vector.scalar_tensor_tensor(rt[:], sums[:], corrt[:], sob,
                                               ALU.subtract, ALU.mult)
                nc.vector.reciprocal_approx_fast(rt[:], rt[:])

                # requant: per-b (et*r) min 127 -> int16 (2x mode), then one
                # merged int32 (hi<<8)|lo pack -> each word holds 4 int8 lanes
                q16 = qpool.tile([P, B * W], mybir.dt.int16, tag="q16")
                for b in range(B):
                    nc.vector.tensor_scalar(q16[:, b * W:(b + 1) * W],
                                            et[:, b * W:(b + 1) * W],
                                            rt[:, b:b + 1], 127.0,
                                            ALU.mult, ALU.min)
                hw = W // 2
                yt = ypool.tile([P, B * hw], mybir.dt.int16, tag="yt")
                q32 = q16[:, :].bitcast(mybir.dt.int32)
                _stt_shift_or(
                    nc, yt[:, :].bitcast(mybir.dt.int32),
                    _ap3(q32, hw // 2, W // 2, B, hw // 2),
                    _ap3(q32, 0, W // 2, B, hw // 2))

                nc.sync.dma_start(
                    y[off // 2:off // 2 + P * B * hw].rearrange(
                        "(p n) -> p n", p=P), yt[:])
    if compile:
        nc.compile()
    return nc


_tril_mask = None


def _host_prep(x_q, scale_x, scale_out):
    global _tril_mask
    x_q = np.asarray(x_q)
    assert x_q.dtype == np.int8, x_q.dtype
    scale_x = np.asarray(scale_x, dtype=np.float32).reshape(H, S)
    scale_out = np.asarray(scale_out, dtype=np.float32).reshape(H, S)

    if _tril_mask is None:
        _tril_mask = np.tril(np.ones((S, S), dtype=np.int8))
    x_pm = x_q * _tril_mask  # zero the strict upper triangle

    # [P, H, NT]: sxr[p, h, t] = scale_x[h, t*128 + p]
    sxr = scale_x.reshape(H, NT, P).transpose(2, 0, 1)
    sor = scale_out.reshape(H, NT, P).transpose(2, 0, 1)

    corr = (127 - np.arange(P)).astype(np.float32).reshape(P, 1)
    tri = np.tril(np.ones((P, P), dtype=np.float16))

    in_maps = []
    for c in range(NCORES):
        xc = np.empty(TOTAL, np.int8)
        for h in range(HPC):
            hg = c * HPC + h
            for t in range(NT):
                off, W = _BLK[h][t]
                # [B, P, W] -> [P, B, W] flattened
                blk = x_pm[:, hg, t * P:(t + 1) * P, 0:W].transpose(1, 0, 2)
                xc[off:off + P * B * W] = blk.reshape(-1)
        hs = slice(c * HPC, (c + 1) * HPC)
        sxc = np.ascontiguousarray(sxr[:, hs].reshape(P, HPC * NT))
        soc = np.ascontiguousarray(sor[:, hs].reshape(P, HPC * NT))
        in_maps.append({"x": xc, "sx": sxc, "so": soc, "corr": corr, "tri": tri})
    return in_maps


def _host_unpack(results):
    out = np.zeros((B, H, S, S), np.int8)
    for c in range(NCORES):
        yc = np.asarray(results[c]["y"]).view(np.int8)  # [TOTAL] bytes
        for h in range(HPC):
            hg = c * HPC + h
            for t in range(NT):
                off, W = _BLK[h][t]
                hw = W // 2
                # int32 words: bytes [y[2g], y[hw+2g], y[2g+1], y[hw+2g+1]]
                blk = yc[off:off + P * B * W].reshape(P, B, W // 4, 4)
                tp = blk.transpose(1, 0, 2, 3)
                dst = out[:, hg, t * P:(t + 1) * P, 0:W]
                dst[:, :, 0:hw:2] = tp[:, :, :, 0]
                dst[:, :, hw::2] = tp[:, :, :, 1]
                dst[:, :, 1:hw:2] = tp[:, :, :, 2]
                dst[:, :, hw + 1::2] = tp[:, :, :, 3]
    return out


def run(x_q, scale_x, scale_out, trace=False):
    global _cached_nc
    if trace:
        _ensure_ntff_hook()
    if _cached_nc is None:
        _cached_nc = _build_bass()
    in_maps = _host_prep(x_q, scale_x, scale_out)
    res = run_bass_kernel_spmd(_cached_nc, in_maps, core_ids=list(range(NCORES)),
                               trace=trace)
    return _host_unpack(res.results), res


def kernel(x_q, scale_x, scale_out):
    out, _ = run(x_q, scale_x, scale_out,
                 trace=bool(int(os.environ.get("KERNEL_TRACE", "0"))))
    return out
